# revision 20
# baseline (speedup 1.0000x reference)
"""MoE block (B=16, C=192, H=W=32, E=8, top-2, 3x3 same-conv experts) on 8 trn2 cores.

Strategy (v2 — Winograd):
  - Router + top-2 combine computed on host; conv is linear in weights, so
    each sample gets ONE host-combined 3x3 conv (2 samples per core).
  - F(2x2, 3x3) Winograd: host precomputes the input transform V = B^T d B
    (bf16) and the weight transform U = G g G^T per sample.
  - The output ROW transform (A^T, rows of the 4x4 position grid) is folded
    into PSUM accumulation: for each output-row-parity i and column-position
    v, plane P[i][v] = sum_u A^T[i,u] M[u][v] accumulates directly via
    matmuls whose lhsT carries the +-U signs.
  - K-perfect chunking: the 4 positions x 192 channels = 768 contraction rows
    per (v) are stored as ONE SBUF "super-stack" [128 x 6 chunks x 256 tiles];
    P[0] consumes rows 0..575, P[1] rows 192..767 via partition-offset
    matmuls.  10 matmuls of N=256 per (i, v, M-chunk pair) -> 20480 PE rows
    per sample (vs 36864 for the direct 9-tap conv).
  - Device output COLUMN transform on DVE: Y[i][0] = P0+P1+P2,
    Y[i][1] = P1-P2-P3, with the per-channel bias folded in through
    scalar_tensor_tensor's per-partition scalar operand.
  - Outputs leave as bf16 planes [s, i, o, (j, tile)]; host reassembles and
    upcasts to fp32.
"""

import numpy as np

B, C, H, W = 16, 192, 32, 32
E, TOPK = 8, 2
NCORES = 8
S = B // NCORES          # samples per core
NT = 16                  # output tiles per spatial dim
TILES = NT * NT          # 256 = matmul moving dim
N_WARMUP = 10

_cache = {}

# F(2x2, 3x3) transform matrices
_G = np.array([[1, 0, 0], [0.5, 0.5, 0.5], [0.5, -0.5, 0.5], [0, 0, 1]], np.float32)
_Bt = np.array([[1, 0, -1, 0], [0, 1, 1, 0], [0, -1, 1, 0], [0, 1, 0, -1]], np.float32)

# per-(i, v) matmul chunk plans: (slot in ust tile, part-slice, v-chunk in vstk)
# U stack layout (8 slots of 128 rows): [U0, U1, U2 | U2hi, pad | -U2, -U3]
# rows 0..575 = +U0,+U1,+U2 (i=0); slot4[64:] pad; slots 5..7 = -U2,-U3
# re-aligned so each slot k pairs with V super-stack chunk at the same
# partition offsets.
_FULL = (0, 128)
_LO = (0, 64)
_HI = (64, 128)
_PLAN = {
    0: [(0, _FULL, 0), (1, _FULL, 1), (2, _FULL, 2), (3, _FULL, 3), (4, _LO, 4)],
    1: [(1, _HI, 1), (2, _FULL, 2), (5, _FULL, 3), (6, _FULL, 4), (7, _FULL, 5)],
}


def _build_module():
    import concourse.tile as tile
    from concourse import bacc, mybir

    f32 = mybir.dt.float32
    bf16 = mybir.dt.bfloat16
    add = mybir.AluOpType.add
    sub = mybir.AluOpType.subtract

    nc = bacc.Bacc("TRN2", target_bir_lowering=False, debug=False, num_devices=NCORES)
    vstk_d = nc.dram_tensor("vstk", [S, 4, 128, 6, TILES], bf16, kind="ExternalInput")
    ustk_d = nc.dram_tensor("ustk", [S, 4, 128, 8, C], bf16, kind="ExternalInput")
    bias_d = nc.dram_tensor("bias", [128, 2 * S], f32, kind="ExternalInput")
    out_d = nc.dram_tensor("out", [S, 2, 128, 4, TILES], bf16,
                           kind="ExternalOutput")

    with tile.TileContext(nc) as tc:
        with (
            tc.tile_pool(name="vin", bufs=1) as vin,
            tc.tile_pool(name="uin", bufs=1) as uin,
            tc.tile_pool(name="cst", bufs=1) as cst,
            tc.tile_pool(name="psA", bufs=2, space="PSUM") as psA,
            tc.tile_pool(name="psB", bufs=2, space="PSUM") as psB,
            tc.tile_pool(name="tsc", bufs=2) as tsc,
            tc.tile_pool(name="yout", bufs=2) as yout,
        ):
            # PE warmup while input DMAs stream in.
            scr = cst.tile([128, TILES], bf16, name="scr", tag="scr")
            nc.vector.memset(scr[:], 0.0)
            ps_scr = psA.tile([128, 2, TILES], f32, name="ps_scr", tag="psAa")
            for _ in range(N_WARMUP):
                nc.tensor.matmul(ps_scr[:, 0, :], scr[:, 0:128], scr[:],
                                 start=True, stop=True, skip_group_check=True)

            bias_t = cst.tile([128, 2 * S], f32, name="bias_t", tag="bias_t")

            VT = {}
            UT = {}

            def emit_input_dmas(s):
                # All input DMAs on ONE queue (sync) in exact consumption
                # order, so the serialized DMA-engine track never runs a
                # far-future transfer while a near-term one waits.
                for v in range(4):
                    vt = vin.tile([128, 6, TILES], bf16, name=f"vt{s}{v}",
                                  tag=f"vt{s}{v}")
                    ut = uin.tile([128, 8, C], bf16, name=f"ut{s}{v}",
                                  tag=f"ut{s}{v}")
                    if s == 0 and v == 0:
                        # low-latency head: V00 via Pool SWDGE (bypasses the
                        # serialized HWDGE issue queue); U00 split on sync
                        nc.gpsimd.dma_start(vt[:], vstk_d[s, v])
                        nc.sync.dma_start(ut[:, 0:3, :], ustk_d[s, v, :, 0:3])
                        nc.sync.dma_start(ut[:, 3:8, :], ustk_d[s, v, :, 3:8])
                    elif s == S - 1 and v == 3:
                        # fine-grained interleave so the final matmuls start
                        # on partial data; the last tiny piece (U slot 7)
                        # feeds only the final chunk-pair of matmuls
                        nc.sync.dma_start(vt[:, 0:2, :], vstk_d[s, v, :, 0:2])
                        nc.sync.dma_start(ut[:, 0:3, :], ustk_d[s, v, :, 0:3])
                        nc.sync.dma_start(vt[:, 2:4, :], vstk_d[s, v, :, 2:4])
                        nc.sync.dma_start(ut[:, 3:6, :], ustk_d[s, v, :, 3:6])
                        nc.sync.dma_start(vt[:, 4:5, :], vstk_d[s, v, :, 4:5])
                        nc.sync.dma_start(ut[:, 6:7, :], ustk_d[s, v, :, 6:7])
                        nc.sync.dma_start(vt[:, 5:6, :], vstk_d[s, v, :, 5:6])
                        nc.sync.dma_start(ut[:, 7:8, :], ustk_d[s, v, :, 7:8])
                    else:
                        nc.sync.dma_start(vt[:], vstk_d[s, v])
                        nc.sync.dma_start(ut[:], ustk_d[s, v])
                    VT[(s, v)] = vt
                    UT[(s, v)] = ut
                if s == 0:
                    nc.scalar.dma_start(bias_t[:], bias_d[:])

            def emit_sample(s):
                """Per v: matmuls for group i then that group's pipelined
                column-transform step (Y0 = P0+P1+P2+b after v2,
                Y1 = P1-P2-P3+b after v3), so DVE work for i0 overlaps PE
                work for i1 and only y1 + a small out-DMA trail the last
                matmul.  PSUM planes are per-v tiles so later-v matmuls never
                wait on earlier-v plane readers.  ACT evicts P1 with bias
                folded (DVE may read only one PSUM operand per op).  s0's
                out-DMAs are deferred past the last input transfer so they
                never steal serialized DMA-track time from input streaming."""
                last = s == S - 1
                # v-planes paired (v0,v3) / (v1,v2) per PSUM tile: later-v
                # matmul writes never share a tile with still-pending earlier-v
                # plane readers, so accumulation never stalls on the column
                # transform.
                _pa = {i: psA.tile([128, 2, TILES], f32, name=f"pa_{s}{i}",
                                   tag="psAa") for i in range(2)}
                _pb = {i: psA.tile([128, 2, TILES], f32, name=f"pb_{s}{i}",
                                   tag="psAb") for i in range(2)}
                _qa = {i: psB.tile([64, 2, TILES], f32, name=f"qa_{s}{i}",
                                   tag="psBa") for i in range(2)}
                _qb = {i: psB.tile([64, 2, TILES], f32, name=f"qb_{s}{i}",
                                   tag="psBb") for i in range(2)}
                _VMAP = {0: (0, 0), 3: (0, 1), 1: (1, 0), 2: (1, 1)}

                def _plane(group, i, v):
                    t, idx = _VMAP[v]
                    tiles = (_pa, _pb) if group == 0 else (_qa, _qb)
                    return tiles[t][i][:, idx, :]

                pm0 = {(i, v): _plane(0, i, v) for i in range(2) for v in range(4)}
                pm1 = {(i, v): _plane(1, i, v) for i in range(2) for v in range(4)}
                # one output tile per column-parity j, holding all four
                # (i, M-chunk) pieces -> a single out-DMA per (sample, j)
                ymj = {j: yout.tile([128, 4, TILES], bf16, name=f"ym{s}{j}",
                                    tag=f"ymj{j}") for j in range(2)}
                p1b, t0, t1 = {}, {}, {}

                def quads(i, v):
                    # (pm plane, n_parts, bias col, ym col) per M-chunk
                    return ((pm0[(i, v)], 128, 2 * s, i),
                            (pm1[(i, v)], 64, 2 * s + 1, 2 + i))


                for v in range(4):
                    vt, ut = VT[(s, v)], UT[(s, v)]
                    for i in range(2):
                        plan = _PLAN[i]
                        for mlo, mhi, pms in ((0, 128, pm0), (128, 192, pm1)):
                            for idx, (slot, (p0, p1), vc) in enumerate(plan):
                                nc.tensor.matmul(
                                    pms[(i, v)], ut[p0:p1, slot, mlo:mhi],
                                    vt[p0:p1, vc, :],
                                    start=(idx == 0), stop=(idx == len(plan) - 1))
                        if v == 1:
                            for k, (pm, np_, bcol, yc) in enumerate(quads(i, 1)):
                                pb = tsc.tile([np_, TILES], bf16,
                                              name=f"p1b_{s}{i}{k}", tag=f"p1b{i}{k}")
                                nc.scalar.activation(
                                    pb[:], pm,
                                    mybir.ActivationFunctionType.Identity,
                                    bias=bias_t[0:np_, bcol:bcol + 1], scale=1.0)
                                p1b[(i, k)] = pb
                                tt = tsc.tile([np_, TILES], bf16,
                                              name=f"t0_{s}{i}{k}", tag=f"t0{i}{k}")
                                nc.vector.tensor_tensor(
                                    tt[:], pb[:], pm0[(i, 0)] if np_ == 128
                                    else pm1[(i, 0)], add)
                                t0[(i, k)] = tt
                        elif v == 2:
                            for k, (pm, np_, bcol, yc) in enumerate(quads(i, 2)):
                                p2 = tsc.tile([np_, TILES], bf16,
                                              name=f"p2_{s}{i}{k}", tag=f"p2{i}{k}")
                                nc.scalar.activation(
                                    p2[:], pm,
                                    mybir.ActivationFunctionType.Identity,
                                    bias=0.0, scale=1.0)
                                nc.vector.tensor_tensor(
                                    ymj[0][0:np_, yc, :], t0[(i, k)][:], p2[:], add)
                                tt = tsc.tile([np_, TILES], bf16,
                                              name=f"t1_{s}{i}{k}", tag=f"t1{i}{k}")
                                nc.vector.tensor_tensor(
                                    tt[:], p1b[(i, k)][:], p2[:], sub)
                                t1[(i, k)] = tt
                            if i == 1:
                                # all four j=0 pieces are in; ship plane j=0
                                eng = nc.scalar if last else nc.sync
                                eng.dma_start(out_d[s, 0, :, 0:2], ymj[0][:, 0:2, :])
                                eng.dma_start(out_d[s, 0, 0:64, 2:4],
                                              ymj[0][0:64, 2:4, :])
                        elif v == 3:
                            for k, (pm, np_, bcol, yc) in enumerate(quads(i, 3)):
                                nc.vector.tensor_tensor(
                                    ymj[1][0:np_, yc, :], t1[(i, k)][:], pm, sub)
                            if i == 1:
                                eng = nc.sync
                                eng.dma_start(out_d[s, 1, :, 0:2], ymj[1][:, 0:2, :])
                                eng.dma_start(out_d[s, 1, 0:64, 2:4],
                                              ymj[1][0:64, 2:4, :])

            emit_input_dmas(0)
            emit_input_dmas(1)
            for s in range(S):
                emit_sample(s)

    nc.compile()
    return nc


def get_module():
    if "nc" not in _cache:
        _cache["nc"] = _build_module()
    return _cache["nc"]


def _route(x, gate_w, gate_b):
    """Replicates the reference router in numpy fp32. Returns combine [B,E]."""
    pooled = x.mean(axis=(2, 3), dtype=np.float32)
    logits = pooled @ gate_w + gate_b
    z = logits - logits.max(axis=-1, keepdims=True)
    ez = np.exp(z)
    w = ez / ez.sum(axis=-1, keepdims=True)
    topi = np.argsort(-w, axis=-1, kind="stable")[:, :TOPK]
    topw = np.take_along_axis(w, topi, axis=-1)
    topw = topw / (topw.sum(-1, keepdims=True) + 1e-10)
    combine = np.zeros((B, E), np.float32)
    np.put_along_axis(combine, topi, topw, axis=-1)
    return combine


def make_in_maps(x, gate_w, gate_b, expert_w, expert_b):
    import ml_dtypes
    bf16 = ml_dtypes.bfloat16

    x = np.ascontiguousarray(np.asarray(x, np.float32))
    gate_w = np.asarray(gate_w, np.float32)
    gate_b = np.asarray(gate_b, np.float32)
    expert_w = np.asarray(expert_w, np.float32)
    expert_b = np.asarray(expert_b, np.float32)

    combine = _route(x, gate_w, gate_b)                       # [B,E]
    Wc = np.einsum("be,eoikl->boikl", combine, expert_w)      # [B,C,C,3,3]
    bc = combine @ expert_b                                   # [B,C]

    # Weight transform U[b,u,v,c,o] (lhsT layout: K=in-ch, M=out-ch)
    U = np.einsum("uk,bockl,vl->buvco", _G, Wc, _G)

    # Input transform V[b,u,v,c,r,t]
    xp = np.zeros((B, C, H + 2, W + 2), np.float32)
    xp[:, :, 1:H + 1, 1:W + 1] = x
    d = np.empty((B, 4, 4, C, NT, NT), np.float32)
    for u in range(4):
        for v in range(4):
            d[:, u, v] = xp[:, :, u:u + 2 * NT:2, v:v + 2 * NT:2]
    V = np.einsum("uk,bklcrt,vl->buvcrt", _Bt, d, _Bt)

    # V super-stack [b, v, row=(u*C+c), tile] -> [b, v, part, chunk, tile]
    vstk = V.transpose(0, 2, 1, 3, 4, 5).reshape(B, 4, 4 * C, TILES)
    vstk = vstk.reshape(B, 4, 6, 128, TILES).transpose(0, 1, 3, 2, 4)
    vstk = np.ascontiguousarray(vstk.astype(bf16))            # [B,4,128,6,T]

    # U stacks with A^T row-fold signs baked in. ust[b, v, slot, part, o]
    # Slots: 0..3 = +U0,+U1,+U2 rows 0..511; slot4[0:64] = +U2 rows 512..575;
    # slots 5..7 = [-U2, -U3] rows re-aligned to V chunk partition offsets.
    Uall = U.transpose(0, 2, 1, 3, 4).reshape(B, 4, 4 * C, C)  # rows (u, c)
    ust = np.zeros((B, 4, 8, 128, C), np.float32)
    for k in range(4):
        ust[:, :, k] = Uall[:, :, 128 * k:128 * (k + 1)]
    ust[:, :, 4, 0:64] = Uall[:, :, 512:576]
    ust[:, :, 5] = -Uall[:, :, 384:512]
    ust[:, :, 6] = -Uall[:, :, 512:640]
    ust[:, :, 7] = -Uall[:, :, 640:768]
    ust = np.ascontiguousarray(ust.transpose(0, 1, 3, 2, 4).astype(bf16))
    # [B, 4, 128, 8, C]

    in_maps = []
    for c in range(NCORES):
        b0 = S * c
        bias = np.zeros((128, 2 * S), np.float32)
        for s in range(S):
            bias[:, 2 * s] = bc[b0 + s, 0:128]
            bias[0:64, 2 * s + 1] = bc[b0 + s, 128:192]
        in_maps.append({
            "vstk": np.ascontiguousarray(vstk[b0:b0 + S]),
            "ustk": np.ascontiguousarray(ust[b0:b0 + S]),
            "bias": bias,
        })
    return in_maps


def unshard_core(out_arr):
    """[S, 2(j), 128, 4(col), TILES] bf16 -> [S, C, H, W] fp32 for one core.
    col 0/1 = out-ch 0..127 for i=0/1; col 2/3 = out-ch 128..191 (parts<64)."""
    a = np.asarray(out_arr, np.float32).reshape(S, 2, 128, 4, NT, NT)
    full = np.empty((S, C, H, W), np.float32)
    for i in range(2):
        for j in range(2):
            full[:, 0:128, i::2, j::2] = a[:, j, :, i]
            full[:, 128:192, i::2, j::2] = a[:, j, 0:64, 2 + i]
    return full


def kernel(x, gate_w, gate_b, expert_w, expert_b):
    from concourse.bass_utils import run_bass_kernel_spmd

    nc = get_module()
    in_maps = make_in_maps(x, gate_w, gate_b, expert_w, expert_b)
    res = run_bass_kernel_spmd(nc, in_maps, core_ids=list(range(NCORES)))
    out = np.concatenate(
        [unshard_core(res.results[c]["out"]) for c in range(NCORES)])
    return out.reshape(B, C, H, W)
